# revision 2
# baseline (speedup 1.0000x reference)
"""Trainium2 Bass kernel for nn_CrossAttention (gnn_message_passing), v3.

Math (per batch b):
  q = (q_w/sqrt(D)) @ pcd            (C, N)
  k = k_w @ neighbors                (C, N, K)
  v = v_w @ neighbors                (C, N, K)
  energy[h,n,k] = sum_d q[h*8+d,n] * k[h*8+d,n,k]
  attn = softmax_k(energy)
  x[h*8+d, n] = sum_k attn[h,n,k] * v[h*8+d,n,k]

v2 redesign vs v1: every stage-2 tensor uses a (chunk, k, n-inner) free
layout so the q-broadcast and all tree adds have step-1 innermost dims ->
DVE 2x_1p mode everywhere operands are bf16/SBUF.  The attn and attn*v
tensors share one "duo" tile so both K-reduction trees run as single fused
DVE ops.  ScalarE evicts v always and k on EVICT_K of every 4 strips
(balancing ACT vs DVE); the remaining strips read k straight from PSUM.
"""

import numpy as np
import ml_dtypes

import concourse.bass as bass
import concourse.tile as tile
from concourse import bacc, mybir
from concourse.bass_utils import run_bass_kernel_spmd

BF16 = mybir.dt.bfloat16
F32 = mybir.dt.float32

B, CIN, N, K = 4, 64, 8192, 32
COUT, H = 64, 8
D = COUT // H
NCORES = 8
NC = N // NCORES  # points per core

_nc_cache = {}


def build_nc(NC=NC, R=1, evict_k=3, psum_bufs=3, fin16=False):
    """Per-core program.  evict_k: of every 16 strips, how many get an ACT
    k-eviction (the rest read k from PSUM on the DVE at 1x).  fin16: keep
    the per-SB finals (level-1 sums, reciprocal, normalize) in bf16."""
    key = (NC, R, evict_k, psum_bufs, fin16)
    if key in _nc_cache:
        return _nc_cache[key]

    S = 32           # points per strip
    PTS = 16         # points per matmul chunk (1 PSUM bank = 32*16 fp32)
    NJ = S // PTS    # chunks per strip
    SB = 128         # normalization batch (points)
    assert NC % SB == 0 and SB % S == 0

    nc = bacc.Bacc("TRN2", target_bir_lowering=False, debug=False,
                   num_devices=NCORES)
    nbp = nc.dram_tensor("nbp", [2, 128, NC // S, NJ, K, PTS], BF16,
                     kind="ExternalInput").ap()
    pcdp = nc.dram_tensor("pcdp", [2, 128, NC], BF16, kind="ExternalInput").ap()
    wq_d = nc.dram_tensor("wq", [128, 128], BF16, kind="ExternalInput").ap()
    wk_d = nc.dram_tensor("wk", [128, 128], BF16, kind="ExternalInput").ap()
    wv_d = nc.dram_tensor("wv", [128, 128], BF16, kind="ExternalInput").ap()
    wo_d = nc.dram_tensor("wones", [128, 128], BF16, kind="ExternalInput").ap()
    xout = nc.dram_tensor("xout", [2, 128, NC], F32, kind="ExternalOutput").ap()

    Exp = mybir.ActivationFunctionType.Exp

    with tile.TileContext(nc) as tc:
        with (
            tc.tile_pool(name="const", bufs=1) as cpool,
            tc.tile_pool(name="io", bufs=6) as iopool,
            tc.tile_pool(name="work", bufs=2) as wpool,
            tc.tile_pool(name="work3", bufs=3) as wpool3,
            tc.tile_pool(name="acc", bufs=2) as apool,
            tc.tile_pool(name="ps", bufs=psum_bufs, space="PSUM") as pspool,
        ):
            wq_t = cpool.tile([128, 128], BF16, tag="wq")
            nc.sync.dma_start(wq_t[:], wq_d[:])
            wk_t = cpool.tile([128, 128], BF16, tag="wk")
            nc.sync.dma_start(wk_t[:], wk_d[:])
            wv_t = cpool.tile([128, 128], BF16, tag="wv")
            nc.sync.dma_start(wv_t[:], wv_d[:])
            wo_t = cpool.tile([128, 128], BF16, tag="wo")
            nc.sync.dma_start(wo_t[:], wo_d[:])

            def body():
                NSP = NC // S       # strips per pp
                NST = 2 * NSP       # total strips (flat over both pp)
                PF = 4              # nb prefetch depth (strips)
                QCH = 512
                nb_tiles = {}
                q_sbs = {}
                x_strips = {}
                t4_box = [None]

                def load_nb(s):
                    pp, si = divmod(s, NSP)
                    nbt = iopool.tile([128, NJ, K, PTS], BF16, tag="nb")
                    nc.sync.dma_start(nbt[:], nbp[pp, :, si])
                    nb_tiles[s] = nbt

                def qproj(pp):
                    # pcd DMA + q projection -> q_sb bf16 (ScalarE eviction)
                    pcd_t = iopool.tile([128, NC], BF16, tag="pcd")
                    nc.sync.dma_start(pcd_t[:], pcdp[pp])
                    q_sb = apool.tile([128, NC], BF16, tag="q", name="q_sb")
                    for h in range(NC // QCH):
                        q_ps = pspool.tile([128, NJ, K, PTS], F32, tag="ps")
                        qp = q_ps[:, 0].rearrange("p a b -> p (a b)")
                        nc.tensor.matmul(qp, wq_t[:],
                                         pcd_t[:, h * QCH:(h + 1) * QCH],
                                         start=True, stop=True)
                        nc.scalar.add(q_sb[:, h * QCH:(h + 1) * QCH], qp, 0.0)
                    q_sbs[pp] = q_sb
                    x_strips[pp] = apool.tile([128, NC], F32, tag="xs",
                                              name="x_strip")

                def front(s):
                    # PE projections + ACT evictions for strip s
                    n0 = (s * S) % NC
                    nbt = nb_tiles.pop(s)
                    # moving APs: pretiled flat chunks (k outer, n inner)
                    mv = [nbt[:, j] for j in range(NJ)]
                    v_ps = pspool.tile([128, NJ, K, PTS], F32, tag="ps")
                    for j in range(NJ):
                        nc.tensor.matmul(v_ps[:, j], wv_t[:], mv[j],
                                         start=True, stop=True)
                    v_sb = wpool3.tile([128, NJ, K, PTS], BF16, tag="vsb")
                    nc.scalar.add(v_sb[:], v_ps[:], 0.0)
                    k_ps = pspool.tile([128, NJ, K, PTS], F32, tag="ps")
                    for j in range(NJ):
                        nc.tensor.matmul(k_ps[:, j], wk_t[:], mv[j],
                                         start=True, stop=True)
                    # Bresenham-spread eviction pattern: evict_k of every 16
                    # strips get an ACT k-eviction, evenly interleaved.
                    if ((s + 1) * evict_k) // 16 > (s * evict_k) // 16:
                        k_src = wpool.tile([128, NJ, K, PTS], BF16,
                                           tag="ksb")
                        nc.scalar.add(k_src[:], k_ps[:], 0.0)
                    else:
                        k_src = k_ps
                    return (v_sb, k_src, s)

                def mid1(st):
                    # DVE q*k, PE e-matmuls for strip s
                    v_sb, k_src, s = st
                    n0 = (s * S) % NC
                    qb = (q_sbs[s // NSP][:, n0:n0 + S]
                          .rearrange("p (j n) -> p j n", j=NJ)
                          .unsqueeze(2).broadcast_to([128, NJ, K, PTS]))
                    prod = wpool.tile([128, NJ, K, PTS], BF16, tag="prod")
                    nc.vector.tensor_mul(prod[:], k_src[:], qb)
                    e_ps = pspool.tile([128, NJ, K, PTS], F32, tag="ps")
                    for j in range(NJ):
                        nc.tensor.matmul(e_ps[:, j], wo_t[:], prod[:, j],
                                         start=True, stop=True)
                    return (v_sb, e_ps, s)

                def mid2(st):
                    # ACT exp for strip s (emitted first in an iteration
                    # so the ready exp never queues behind evictions)
                    v_sb, e_ps, s = st
                    duo = wpool3.tile([128, 2, NJ, K, PTS], BF16,
                                      tag="duo")
                    nc.scalar.activation(duo[:, 0], e_ps[:], Exp)
                    return (v_sb, duo, s)

                def back(st):
                    # DVE attn*v, fused trees for both sums, per-SB finals
                    v_sb, duo, s = st
                    n0 = (s * S) % NC
                    x_strip = x_strips[s // NSP]
                    if n0 % SB == 0:
                        t4 = wpool.tile([128, 2, SB // PTS, 4, PTS],
                                        BF16, tag="t4", name="t4")
                        t4_box[0] = t4
                    t4 = t4_box[0]
                    nc.vector.tensor_mul(duo[:, 1], duo[:, 0], v_sb[:])
                    l16 = wpool.tile([128, 2, NJ, 16, PTS], BF16, tag="l16")
                    nc.vector.tensor_add(l16[:], duo[:, :, :, 0:16, :],
                                         duo[:, :, :, 16:32, :])
                    l8 = wpool.tile([128, 2, NJ, 8, PTS], BF16, tag="l8")
                    nc.vector.tensor_add(l8[:], l16[:, :, :, 0:8, :],
                                         l16[:, :, :, 8:16, :])
                    g0 = (n0 % SB) // PTS
                    nc.vector.tensor_add(t4[:, :, g0:g0 + NJ, :, :],
                                         l8[:, :, :, 0:4, :],
                                         l8[:, :, :, 4:8, :])
                    if (n0 + S) % SB == 0:
                        nb0 = n0 + S - SB
                        l2 = wpool.tile([128, 2, SB // PTS, 2, PTS], BF16,
                                        tag="l2")
                        nc.vector.tensor_add(l2[:], t4[:, :, :, 0:2, :],
                                             t4[:, :, :, 2:4, :])
                        l1 = wpool.tile([128, 2, SB // PTS, PTS], F32,
                                        tag="l1")
                        nc.vector.tensor_add(l1[:], l2[:, :, :, 0, :],
                                             l2[:, :, :, 1, :])
                        rden = wpool.tile([128, SB // PTS, PTS], F32,
                                          tag="rden")
                        nc.vector.reciprocal(rden[:], l1[:, 0])
                        xsv = x_strip[:, nb0:nb0 + SB].rearrange(
                            "p (g n) -> p g n", g=SB // PTS)
                        nc.vector.tensor_mul(xsv, l1[:, 1], rden[:])
                    if (s + 1) % NSP == 0:
                        nc.sync.dma_start(xout[s // NSP], x_strip[:])

                # software pipeline per iteration i (flat over both pp):
                #   exp(i-2) | front(i) | q*k+e-mm(i-1) | back(i-2)
                qproj(0)
                for s in range(min(PF, NST)):
                    load_nb(s)
                p_front = p_mid1 = None
                for s in range(NST):
                    if s + PF < NST:
                        load_nb(s + PF)
                    if s == NSP - 6:
                        qproj(1)
                    st2 = mid2(p_mid1) if p_mid1 is not None else None
                    st = front(s)
                    new_mid1 = (mid1(p_front) if p_front is not None
                                else None)
                    if st2 is not None:
                        back(st2)
                    p_front, p_mid1 = st, new_mid1
                back(mid2(p_mid1))
                back(mid2(mid1(p_front)))

            if R == 1:
                body()
            elif R < 0:
                for _ in range(-R):  # python-unrolled (for TimelineSim)
                    body()
            else:
                with tc.For_i(0, R, 1):
                    body()

    nc.compile()
    _nc_cache[key] = nc
    return nc


def prep_inputs(pcd, neighbors, q_w, k_w, v_w, NC=NC):
    """Host-side prep: cast to bf16, pair-stack batches, build stationaries."""
    bf = ml_dtypes.bfloat16
    s = 1.0 / np.sqrt(np.float32(D))
    qwT = (q_w.astype(np.float32) * s).T.astype(bf)  # (c, hd)
    kwT = k_w.T.astype(bf)
    vwT = v_w.T.astype(bf)

    def blockdiag(m):
        z = np.zeros((128, 128), dtype=bf)
        z[:64, :64] = m
        z[64:, 64:] = m
        return z

    wq = blockdiag(qwT)
    wk = blockdiag(kwT)
    wv = blockdiag(vwT)
    blk = np.kron(np.eye(H, dtype=np.float32), np.ones((D, D), np.float32))
    wones = blockdiag(blk.astype(bf))

    S, PTS = 32, 16
    nbs = neighbors.reshape(2, 2 * CIN, N, K)    # (pair, bb*64+c, n, k)
    pcds = pcd.reshape(2, 2 * CIN, N)
    ncores = N // NC
    in_maps = []
    for i in range(ncores):
        sl = slice(i * NC, (i + 1) * NC)
        nbc = nbs[:, :, sl, :].reshape(2, 128, NC // S, S // PTS, PTS, K)
        nbc = nbc.transpose(0, 1, 2, 3, 5, 4)    # chunk k-major: (k, nj)
        in_maps.append({
            "nbp": np.ascontiguousarray(nbc).astype(bf),
            "pcdp": np.ascontiguousarray(pcds[:, :, sl]).astype(bf),
            "wq": wq, "wk": wk, "wv": wv, "wones": wones,
        })
    return in_maps


def assemble_output(results, NC=NC):
    ncores = len(results)
    out = np.empty((B, COUT, N), dtype=np.float32)
    for i, r in enumerate(results):
        x = r["xout"].reshape(B, COUT, NC)  # (2,128,NC) -> (4,64,NC)
        out[:, :, i * NC:(i + 1) * NC] = x
    return out


BEST = dict(evict_k=6, psum_bufs=3)


def kernel(pcd, neighbors, q_w, k_w, v_w):
    pcd = np.asarray(pcd, dtype=np.float32)
    neighbors = np.asarray(neighbors, dtype=np.float32)
    nc = build_nc(NC=NC, R=1, **BEST)
    in_maps = prep_inputs(pcd, neighbors, q_w, k_w, v_w)
    res = run_bass_kernel_spmd(nc, in_maps, core_ids=list(range(NCORES)))
    return assemble_output(res.results)


if __name__ == "__main__":
    rng = np.random.default_rng(0)
    ins = {
        "pcd": rng.standard_normal((B, CIN, N), dtype=np.float32),
        "neighbors": rng.standard_normal((B, CIN, N, K), dtype=np.float32),
        "q_w": (rng.standard_normal((COUT, CIN), dtype=np.float32) / 8.0),
        "k_w": (rng.standard_normal((COUT, CIN), dtype=np.float32) / 8.0),
        "v_w": (rng.standard_normal((COUT, CIN), dtype=np.float32) / 8.0),
    }
    out = kernel(**ins)
    print("kernel output", out.shape, out.dtype)


# revision 3
# speedup vs baseline: 1.0931x; 1.0931x over previous
"""Trainium2 Bass kernel for nn_CrossAttention (gnn_message_passing), v3.

Math (per batch b):
  q = (q_w/sqrt(D)) @ pcd            (C, N)
  k = k_w @ neighbors                (C, N, K)
  v = v_w @ neighbors                (C, N, K)
  energy[h,n,k] = sum_d q[h*8+d,n] * k[h*8+d,n,k]
  attn = softmax_k(energy)
  x[h*8+d, n] = sum_k attn[h,n,k] * v[h*8+d,n,k]

v2 redesign vs v1: every stage-2 tensor uses a (chunk, k, n-inner) free
layout so the q-broadcast and all tree adds have step-1 innermost dims ->
DVE 2x_1p mode everywhere operands are bf16/SBUF.  The attn and attn*v
tensors share one "duo" tile so both K-reduction trees run as single fused
DVE ops.  ScalarE evicts v always and k on EVICT_K of every 4 strips
(balancing ACT vs DVE); the remaining strips read k straight from PSUM.
"""

import numpy as np
import ml_dtypes

import concourse.bass as bass
import concourse.tile as tile
from concourse import bacc, mybir
from concourse.bass_utils import run_bass_kernel_spmd

BF16 = mybir.dt.bfloat16
F32 = mybir.dt.float32

B, CIN, N, K = 4, 64, 8192, 32
COUT, H = 64, 8
D = COUT // H
NCORES = 8
NC = N // NCORES  # points per core

_nc_cache = {}


def build_nc(NC=NC, R=1, evict_k=3, psum_bufs=3, dma_frac=4, dma_batch=2):
    """Per-core program.  evict_k: of every 16 strips, how many get an ACT
    k-eviction (the rest read k from PSUM on the DVE at 1x).  dma_frac: of 4,
    fraction of each nb tile actually DMA'd (timing probe; <4 gives wrong
    results).  dma_batch: strips per nb DMA."""
    key = (NC, R, evict_k, psum_bufs, dma_frac, dma_batch)
    if key in _nc_cache:
        return _nc_cache[key]

    S = 32           # points per strip
    PTS = 16         # points per matmul chunk (1 PSUM bank = 32*16 fp32)
    NJ = S // PTS    # chunks per strip
    SB = 128         # normalization batch (points)
    assert NC % SB == 0 and SB % S == 0

    nc = bacc.Bacc("TRN2", target_bir_lowering=False, debug=False,
                   num_devices=NCORES)
    nbp = nc.dram_tensor("nbp", [2, 128, NC // S, NJ, K, PTS], BF16,
                     kind="ExternalInput").ap()
    pcdp = nc.dram_tensor("pcdp", [2, 128, NC], BF16, kind="ExternalInput").ap()
    wq_d = nc.dram_tensor("wq", [128, 128], BF16, kind="ExternalInput").ap()
    wk_d = nc.dram_tensor("wk", [128, 128], BF16, kind="ExternalInput").ap()
    wv_d = nc.dram_tensor("wv", [128, 128], BF16, kind="ExternalInput").ap()
    wo_d = nc.dram_tensor("wones", [128, 128], BF16, kind="ExternalInput").ap()
    xout = nc.dram_tensor("xout", [2, 128, NC], F32, kind="ExternalOutput").ap()

    Exp = mybir.ActivationFunctionType.Exp

    with tile.TileContext(nc) as tc:
        with (
            tc.tile_pool(name="const", bufs=1) as cpool,
            tc.tile_pool(name="io", bufs=6) as iopool,
            tc.tile_pool(name="work", bufs=2) as wpool,
            tc.tile_pool(name="work3", bufs=3) as wpool3,
            tc.tile_pool(name="acc", bufs=2) as apool,
            tc.tile_pool(name="ps", bufs=psum_bufs, space="PSUM") as pspool,
        ):
            wq_t = cpool.tile([128, 128], BF16, tag="wq")
            nc.sync.dma_start(wq_t[:], wq_d[:])
            wk_t = cpool.tile([128, 128], BF16, tag="wk")
            nc.sync.dma_start(wk_t[:], wk_d[:])
            wv_t = cpool.tile([128, 128], BF16, tag="wv")
            nc.sync.dma_start(wv_t[:], wv_d[:])
            wo_t = cpool.tile([128, 128], BF16, tag="wo")
            nc.sync.dma_start(wo_t[:], wo_d[:])

            def body():
                NSP = NC // S       # strips per pp
                NST = 2 * NSP       # total strips (flat over both pp)
                PF = 4              # nb prefetch depth (strips)
                QCH = 512
                nb_tiles = {}
                q_sbs = {}
                x_strips = {}
                t4_box = [None]

                def load_nb(s):
                    if s % dma_batch != 0:
                        return
                    pp, si = divmod(s, NSP)
                    nb2 = min(dma_batch, NSP - si)
                    nbt = iopool.tile([128, dma_batch, NJ, K, PTS], BF16,
                                      tag="nb")
                    if dma_frac >= 4:
                        nc.sync.dma_start(nbt[:, 0:nb2],
                                          nbp[pp, :, si:si + nb2])
                    else:
                        # timing probe: move only a fraction of the bytes
                        nc.sync.dma_start(
                            nbt[:, 0:nb2, :, 0:(K * dma_frac) // 4],
                            nbp[pp, :, si:si + nb2, :, 0:(K * dma_frac) // 4])
                    for u in range(nb2):
                        nb_tiles[s + u] = nbt[:, u]

                def qproj(pp):
                    # pcd DMA + q projection -> q_sb bf16 (ScalarE eviction)
                    pcd_t = iopool.tile([128, NC], BF16, tag="pcd")
                    nc.sync.dma_start(pcd_t[:], pcdp[pp])
                    q_sb = apool.tile([128, NC], BF16, tag="q", name="q_sb")
                    for h in range(NC // QCH):
                        q_ps = pspool.tile([128, NJ, K, PTS], F32, tag="ps")
                        qp = q_ps[:, 0].rearrange("p a b -> p (a b)")
                        nc.tensor.matmul(qp, wq_t[:],
                                         pcd_t[:, h * QCH:(h + 1) * QCH],
                                         start=True, stop=True)
                        nc.scalar.add(q_sb[:, h * QCH:(h + 1) * QCH], qp, 0.0)
                    q_sbs[pp] = q_sb
                    x_strips[pp] = apool.tile([128, NC], F32, tag="xs",
                                              name="x_strip")

                def front(s):
                    # PE projections + ACT evictions for strip s
                    n0 = (s * S) % NC
                    nbt = nb_tiles.pop(s)
                    # moving APs: pretiled flat chunks (k outer, n inner)
                    mv = [nbt[:, j] for j in range(NJ)]
                    v_ps = pspool.tile([128, NJ, K, PTS], F32, tag="ps")
                    for j in range(NJ):
                        nc.tensor.matmul(v_ps[:, j], wv_t[:], mv[j],
                                         start=True, stop=True)
                    v_sb = wpool3.tile([128, NJ, K, PTS], BF16, tag="vsb")
                    nc.scalar.add(v_sb[:], v_ps[:], 0.0)
                    k_ps = pspool.tile([128, NJ, K, PTS], F32, tag="ps")
                    for j in range(NJ):
                        nc.tensor.matmul(k_ps[:, j], wk_t[:], mv[j],
                                         start=True, stop=True)
                    # Bresenham-spread eviction pattern: evict_k of every 16
                    # strips get an ACT k-eviction, evenly interleaved.
                    if ((s + 1) * evict_k) // 16 > (s * evict_k) // 16:
                        k_src = wpool.tile([128, NJ, K, PTS], BF16,
                                           tag="ksb")
                        nc.scalar.add(k_src[:], k_ps[:], 0.0)
                    else:
                        k_src = k_ps
                    return (v_sb, k_src, s)

                def mid1(st):
                    # DVE q*k, PE e-matmuls for strip s
                    v_sb, k_src, s = st
                    n0 = (s * S) % NC
                    qb = (q_sbs[s // NSP][:, n0:n0 + S]
                          .rearrange("p (j n) -> p j n", j=NJ)
                          .unsqueeze(2).broadcast_to([128, NJ, K, PTS]))
                    prod = wpool.tile([128, NJ, K, PTS], BF16, tag="prod")
                    nc.vector.tensor_mul(prod[:], k_src[:], qb)
                    e_ps = pspool.tile([128, NJ, K, PTS], F32, tag="ps")
                    for j in range(NJ):
                        nc.tensor.matmul(e_ps[:, j], wo_t[:], prod[:, j],
                                         start=True, stop=True)
                    return (v_sb, e_ps, s)

                def mid2(st):
                    # ACT exp for strip s (emitted first in an iteration
                    # so the ready exp never queues behind evictions)
                    v_sb, e_ps, s = st
                    duo = wpool3.tile([128, 2, NJ, K, PTS], BF16,
                                      tag="duo")
                    nc.scalar.activation(duo[:, 0], e_ps[:], Exp)
                    return (v_sb, duo, s)

                def back(st):
                    # DVE attn*v, fused trees for both sums, per-SB finals
                    v_sb, duo, s = st
                    n0 = (s * S) % NC
                    x_strip = x_strips[s // NSP]
                    if n0 % SB == 0:
                        t4 = wpool.tile([128, 2, SB // PTS, 4, PTS],
                                        BF16, tag="t4", name="t4")
                        t4_box[0] = t4
                    t4 = t4_box[0]
                    nc.vector.tensor_mul(duo[:, 1], duo[:, 0], v_sb[:])
                    l16 = wpool.tile([128, 2, NJ, 16, PTS], BF16, tag="l16")
                    nc.vector.tensor_add(l16[:], duo[:, :, :, 0:16, :],
                                         duo[:, :, :, 16:32, :])
                    l8 = wpool.tile([128, 2, NJ, 8, PTS], BF16, tag="l8")
                    nc.vector.tensor_add(l8[:], l16[:, :, :, 0:8, :],
                                         l16[:, :, :, 8:16, :])
                    g0 = (n0 % SB) // PTS
                    nc.vector.tensor_add(t4[:, :, g0:g0 + NJ, :, :],
                                         l8[:, :, :, 0:4, :],
                                         l8[:, :, :, 4:8, :])
                    if (n0 + S) % SB == 0:
                        nb0 = n0 + S - SB
                        l2 = wpool.tile([128, 2, SB // PTS, 2, PTS], BF16,
                                        tag="l2")
                        nc.vector.tensor_add(l2[:], t4[:, :, :, 0:2, :],
                                             t4[:, :, :, 2:4, :])
                        l1 = wpool.tile([128, 2, SB // PTS, PTS], F32,
                                        tag="l1")
                        nc.vector.tensor_add(l1[:], l2[:, :, :, 0, :],
                                             l2[:, :, :, 1, :])
                        rden = wpool.tile([128, SB // PTS, PTS], F32,
                                          tag="rden")
                        nc.vector.reciprocal(rden[:], l1[:, 0])
                        xsv = x_strip[:, nb0:nb0 + SB].rearrange(
                            "p (g n) -> p g n", g=SB // PTS)
                        nc.vector.tensor_mul(xsv, l1[:, 1], rden[:])
                    if (s + 1) % NSP == 0:
                        nc.sync.dma_start(xout[s // NSP], x_strip[:])

                # software pipeline per iteration i (flat over both pp):
                #   exp(i-2) | front(i) | q*k+e-mm(i-1) | back(i-2)
                qproj(0)
                for s in range(min(PF, NST)):
                    load_nb(s)
                p_front = p_mid1 = None
                for s in range(NST):
                    if s + PF < NST:
                        load_nb(s + PF)
                    if s == NSP - 6:
                        qproj(1)
                    st2 = mid2(p_mid1) if p_mid1 is not None else None
                    st = front(s)
                    new_mid1 = (mid1(p_front) if p_front is not None
                                else None)
                    if st2 is not None:
                        back(st2)
                    p_front, p_mid1 = st, new_mid1
                back(mid2(p_mid1))
                back(mid2(mid1(p_front)))

            if R == 1:
                body()
            elif R < 0:
                for _ in range(-R):  # python-unrolled (for TimelineSim)
                    body()
            else:
                with tc.For_i(0, R, 1):
                    body()

    nc.compile()
    _nc_cache[key] = nc
    return nc


def prep_inputs(pcd, neighbors, q_w, k_w, v_w, NC=NC):
    """Host-side prep: cast to bf16, pair-stack batches, build stationaries."""
    bf = ml_dtypes.bfloat16
    s = 1.0 / np.sqrt(np.float32(D))
    qwT = (q_w.astype(np.float32) * s).T.astype(bf)  # (c, hd)
    kwT = k_w.T.astype(bf)
    vwT = v_w.T.astype(bf)

    def blockdiag(m):
        z = np.zeros((128, 128), dtype=bf)
        z[:64, :64] = m
        z[64:, 64:] = m
        return z

    wq = blockdiag(qwT)
    wk = blockdiag(kwT)
    wv = blockdiag(vwT)
    blk = np.kron(np.eye(H, dtype=np.float32), np.ones((D, D), np.float32))
    wones = blockdiag(blk.astype(bf))

    S, PTS = 32, 16
    nbs = neighbors.reshape(2, 2 * CIN, N, K)    # (pair, bb*64+c, n, k)
    pcds = pcd.reshape(2, 2 * CIN, N)
    ncores = N // NC
    in_maps = []
    for i in range(ncores):
        sl = slice(i * NC, (i + 1) * NC)
        nbc = nbs[:, :, sl, :].reshape(2, 128, NC // S, S // PTS, PTS, K)
        nbc = nbc.transpose(0, 1, 2, 3, 5, 4)    # chunk k-major: (k, nj)
        in_maps.append({
            "nbp": np.ascontiguousarray(nbc).astype(bf),
            "pcdp": np.ascontiguousarray(pcds[:, :, sl]).astype(bf),
            "wq": wq, "wk": wk, "wv": wv, "wones": wones,
        })
    return in_maps


def assemble_output(results, NC=NC):
    ncores = len(results)
    out = np.empty((B, COUT, N), dtype=np.float32)
    for i, r in enumerate(results):
        x = r["xout"].reshape(B, COUT, NC)  # (2,128,NC) -> (4,64,NC)
        out[:, :, i * NC:(i + 1) * NC] = x
    return out


BEST = dict(evict_k=6, psum_bufs=3)


def kernel(pcd, neighbors, q_w, k_w, v_w):
    pcd = np.asarray(pcd, dtype=np.float32)
    neighbors = np.asarray(neighbors, dtype=np.float32)
    nc = build_nc(NC=NC, R=1, **BEST)
    in_maps = prep_inputs(pcd, neighbors, q_w, k_w, v_w)
    res = run_bass_kernel_spmd(nc, in_maps, core_ids=list(range(NCORES)))
    return assemble_output(res.results)


if __name__ == "__main__":
    rng = np.random.default_rng(0)
    ins = {
        "pcd": rng.standard_normal((B, CIN, N), dtype=np.float32),
        "neighbors": rng.standard_normal((B, CIN, N, K), dtype=np.float32),
        "q_w": (rng.standard_normal((COUT, CIN), dtype=np.float32) / 8.0),
        "k_w": (rng.standard_normal((COUT, CIN), dtype=np.float32) / 8.0),
        "v_w": (rng.standard_normal((COUT, CIN), dtype=np.float32) / 8.0),
    }
    out = kernel(**ins)
    print("kernel output", out.shape, out.dtype)
